# revision 1
# baseline (speedup 1.0000x reference)
"""MLA (multi-head latent attention) prefill block on 8 Trainium2 NeuronCores.

Tensor-parallel over heads: each core computes 4 of the 32 heads end-to-end
(q projection, absorbed q, latent attention, head output projection, and its
partial slice of the output projection). The kv latent path (kv_a projection,
rms-norm, rope) is replicated on every core. Per-core partial outputs (the
row-parallel wo matmul) are summed on the host.

Everything on-device is computed transposed ([feature, seq] layouts) so that
no activation transposes are needed except kv_c / k_pe (done once via the PE
transpose path, shared by all heads).

Self-contained: hardcodes all shapes from the problem spec.
"""

import os
from contextlib import ExitStack

import numpy as np

import concourse.bacc as bacc
import concourse.bass as bass
import concourse.mybir as mybir
import concourse.tile as tile
from concourse.bass_utils import run_bass_kernel_spmd
from concourse.masks import make_identity

# ---- problem constants ----
DIM = 2048
NH = 32
DN = 128  # qk_nope_head_dim
DR = 64   # qk_rope_head_dim
DV = 128  # v_head_dim
KVL = 512  # kv_lora_rank
S = 2048   # sequence length (B=1)
SCALE = float((DN + DR) ** -0.5)
EPS = 1e-6

NCORES = 8
NHC = NH // NCORES      # heads per core = 4
P = 128                 # partitions
SF = 512                # free-dim tile (s tiles)
NST = S // SF           # 4 s tiles
NTT = S // P            # 16 t tiles
NDC = DIM // P          # 16 contraction chunks over model dim
NCC = KVL // P          # 4 latent chunks

F32 = mybir.dt.float32
F32R = mybir.dt.float32r

USE_F32R = os.environ.get("MLA_F32R", "1") == "1"
RT = F32R if USE_F32R else F32  # dtype for all matmul operands


def build_nc(repeat=1):
    """Build the per-core Bass program (identical on all 8 cores)."""
    nc = bacc.Bacc("TRN2", target_bir_lowering=False, debug=False,
                   num_devices=NCORES)

    # ---- DRAM I/O ----
    d_xT = nc.dram_tensor("xT", [DIM, S], RT, kind="ExternalInput")
    d_wqn = nc.dram_tensor("wq_n", [DIM, NHC * DN], RT, kind="ExternalInput")
    d_wqpr = nc.dram_tensor("wq_pr", [DIM, NHC * 32], RT, kind="ExternalInput")
    d_wqpi = nc.dram_tensor("wq_pi", [DIM, NHC * 32], RT, kind="ExternalInput")
    d_wkva = nc.dram_tensor("wkv_a", [DIM, KVL + DR], RT, kind="ExternalInput")
    d_wbk = nc.dram_tensor("wbk", [NHC, DN, KVL], RT, kind="ExternalInput")
    d_wbvT = nc.dram_tensor("wbvT", [NHC, KVL, DV], RT, kind="ExternalInput")
    d_wo = nc.dram_tensor("wo_c", [NHC * DV, DIM], RT, kind="ExternalInput")
    d_cosn = nc.dram_tensor("cos_n", [S, DR // 2], F32, kind="ExternalInput")
    d_sinn = nc.dram_tensor("sin_n", [S, DR // 2], F32, kind="ExternalInput")
    d_cosr = nc.dram_tensor("cosR", [P, S], F32, kind="ExternalInput")
    d_sinr = nc.dram_tensor("sinR", [P, S], F32, kind="ExternalInput")
    d_out = nc.dram_tensor("outT", [DIM, S], F32, kind="ExternalOutput")
    # scratch for q while xT occupies SBUF
    d_qns = nc.dram_tensor("qn_scratch", [NHC, DN, S], RT)
    d_qps = nc.dram_tensor("qp_scratch", [NHC, DR, S], RT)

    xT = d_xT.ap()
    out = d_out.ap()

    with tile.TileContext(nc) as tc:
      for _rep in range(repeat):
        with ExitStack() as top:
            cst = top.enter_context(tc.tile_pool(name="const", bufs=1))
            ident = cst.tile([P, P], F32, tag="ident", name="ident")
            make_identity(nc, ident[:])
            ones_f = cst.tile([P, 1], F32, tag="ones_f", name="ones_f")
            nc.gpsimd.memset(ones_f[:], 1.0)
            ones_c = cst.tile([P, 1], RT, tag="ones_c", name="ones_c")
            nc.scalar.copy(ones_c[:], ones_f[:])
            ones_r = cst.tile([1, P], F32, tag="ones_r", name="ones_r")
            nc.gpsimd.memset(ones_r[:], 1.0)
            epsb = cst.tile([P, 1], F32, tag="epsb", name="epsb")
            nc.gpsimd.memset(epsb[:], EPS)

            # long-lived: normalized latent kv (natural layout)
            kvp = top.enter_context(tc.tile_pool(name="kv", bufs=NTT))
            kvc = [kvp.tile([P, KVL], RT, tag="kvc", name="kvc")
                   for _ in range(NTT)]
            kpp = top.enter_context(tc.tile_pool(name="kpe", bufs=NTT))
            kpe = [kpp.tile([P, DR], F32, tag="kpe", name="kpe")
                   for _ in range(NTT)]

            # ===== phase 1: q + kv projections, single pass over xT ========
            with ExitStack() as ph1:
                wrp = ph1.enter_context(tc.tile_pool(name="wres", bufs=1))
                xsl0 = ph1.enter_context(tc.tile_pool(name="xsl", bufs=6))
                xTj0 = d_xT.ap()[:, 0:SF].rearrange("(d p) f -> p d f", p=P)
                xh0 = [xsl0.tile([P, 4 * SF], RT, tag="xsl", name="xsl")
                       for _ in range(4)]
                wqn_a = wrp.tile([P, NDC * NHC * DN], RT, tag="wqn",
                                 name="wqn")
                wqpr_a = wrp.tile([P, NDC * NHC * 32], RT, tag="wqpr",
                                  name="wqpr")
                wqpi_a = wrp.tile([P, NDC * NHC * 32], RT, tag="wqpi",
                                  name="wqpi")
                wkva_a = wrp.tile([P, NDC * (KVL + DR)], RT, tag="wkva",
                                  name="wkva")
                # interleave x and weight quarters in consumption order so
                # the PE starts after ~2 quarters instead of the full set
                for q4 in range(4):
                    hd = slice(q4 * (NDC // 4), (q4 + 1) * (NDC // 4))
                    nc.sync.dma_start(
                        xh0[q4][:].rearrange("p (d f) -> p d f", d=4),
                        xTj0[:, 4 * q4:4 * (q4 + 1)])
                    nc.sync.dma_start(
                        wqn_a[:].rearrange("p (d c) -> p d c", d=NDC)[:, hd],
                        d_wqn.ap().rearrange("(d p) c -> p d c", p=P)[:, hd])
                    nc.sync.dma_start(
                        wqpr_a[:].rearrange("p (d c) -> p d c", d=NDC)[:, hd],
                        d_wqpr.ap().rearrange("(d p) c -> p d c", p=P)[:, hd])
                    nc.sync.dma_start(
                        wqpi_a[:].rearrange("p (d c) -> p d c", d=NDC)[:, hd],
                        d_wqpi.ap().rearrange("(d p) c -> p d c", p=P)[:, hd])
                    nc.sync.dma_start(
                        wkva_a[:].rearrange("p (d c) -> p d c", d=NDC)[:, hd],
                        d_wkva.ap().rearrange("(d p) c -> p d c", p=P)[:, hd])
                cna = wrp.tile([P, NTT * 32], F32, tag="cna", name="cna")
                sna = wrp.tile([P, NTT * 32], F32, tag="sna", name="sna")
                nc.sync.dma_start(
                    cna[:].rearrange("p (t k) -> p t k", t=NTT),
                    d_cosn.ap().rearrange("(t p) k -> p t k", p=P))
                nc.sync.dma_start(
                    sna[:].rearrange("p (t k) -> p t k", t=NTT),
                    d_sinn.ap().rearrange("(t p) k -> p t k", p=P))

                xsl = xsl0
                stg = ph1.enter_context(tc.tile_pool(name="stg", bufs=1))
                rts = ph1.enter_context(tc.tile_pool(name="ropetmp", bufs=1))
                rox = ph1.enter_context(tc.tile_pool(name="ropeout", bufs=1))
                sqs = ph1.enter_context(tc.tile_pool(name="sqs", bufs=2))
                crs = ph1.enter_context(tc.tile_pool(name="crs", bufs=2))
                kct = ph1.enter_context(tc.tile_pool(name="kct", bufs=2))
                nrm = ph1.enter_context(tc.tile_pool(name="nrm", bufs=4))

                with tc.tile_pool(name="acc1", bufs=8, space="PSUM") as qac:
                    for j in range(NST):
                        js = slice(j * SF, (j + 1) * SF)
                        xTj = d_xT.ap()[:, js].rearrange(
                            "(d p) f -> p d f", p=P)
                        if j == 0:
                            xh = xh0
                        else:
                            xh = [xsl.tile([P, 4 * SF], RT, tag="xsl",
                                           name="xsl") for _ in range(4)]
                            for q4 in range(4):
                                nc.sync.dma_start(
                                    xh[q4][:].rearrange(
                                        "p (d f) -> p d f", d=4),
                                    xTj[:, 4 * q4:4 * (q4 + 1)])
                        # ---- q projections for this s block ----
                        pss = [qac.tile([P, SF], F32, tag="acc", name="acc")
                               for _ in range(NHC + 2)]
                        for d in range(NDC):
                            xs = xh[d // 4][:, (d % 4) * SF:(d % 4 + 1) * SF]
                            for h in range(NHC):
                                nc.tensor.matmul(
                                    pss[h][:],
                                    wqn_a[:, d * NHC * DN + h * DN:
                                          d * NHC * DN + (h + 1) * DN],
                                    xs,
                                    start=(d == 0), stop=(d == NDC - 1))
                            nc.tensor.matmul(
                                pss[NHC][:],
                                wqpr_a[:, d * P:(d + 1) * P], xs,
                                start=(d == 0), stop=(d == NDC - 1))
                            nc.tensor.matmul(
                                pss[NHC + 1][:],
                                wqpi_a[:, d * P:(d + 1) * P], xs,
                                start=(d == 0), stop=(d == NDC - 1))
                        stb = stg.tile([P, NHC * SF], RT, tag="stg",
                                       name="stg")
                        for h in range(NHC):
                            nc.scalar.copy(
                                stb[:, h * SF:(h + 1) * SF], pss[h][:])
                        nc.sync.dma_start(
                            d_qns.ap()[:, :, js].rearrange(
                                "h p f -> p h f"),
                            stb[:].rearrange("p (h f) -> p h f", h=NHC))
                        # rope rotation for q_pe (even=r, odd=i) off PSUM
                        t1 = rts.tile([P, SF], F32, tag="t1", name="t1")
                        t2 = rts.tile([P, SF], F32, tag="t2", name="t2")
                        ror = rox.tile([P, SF], RT, tag="ror", name="ror")
                        roi = rox.tile([P, SF], RT, tag="roi", name="roi")
                        cR = crs.tile([P, SF], F32, tag="cR", name="cR")
                        sR = crs.tile([P, SF], F32, tag="sR", name="sR")
                        nc.sync.dma_start(cR[:], d_cosr.ap()[:, js])
                        nc.sync.dma_start(sR[:], d_sinr.ap()[:, js])
                        nc.vector.tensor_mul(t1[:], pss[NHC][:], cR[:])
                        nc.vector.tensor_mul(t2[:], pss[NHC + 1][:], sR[:])
                        nc.vector.tensor_sub(ror[:], t1[:], t2[:])
                        nc.vector.tensor_mul(t1[:], pss[NHC][:], sR[:])
                        nc.vector.tensor_mul(t2[:], pss[NHC + 1][:], cR[:])
                        nc.vector.tensor_add(roi[:], t1[:], t2[:])
                        for h in range(NHC):
                            hs = slice(h * 32, (h + 1) * 32)
                            nc.sync.dma_start(
                                d_qps.ap()[h, 0:32, js], ror[hs, :])
                            nc.sync.dma_start(
                                d_qps.ap()[h, 32:64, js], roi[hs, :])
                        # ---- kv projection for this t block (same x) ----
                        psc = [qac.tile([P, 320], F32, tag="acc",
                                        name="acc", padded_shape=[P, SF])
                               for _ in range(4)]
                        psp = [qac.tile([P, 256], F32, tag="acc",
                                        name="accp",
                                        padded_shape=[P, SF])
                               for _ in range(4)]
                        for d in range(NDC):
                            xs = xh[d // 4]
                            for ti in range(4):
                                xtsl = xs[:, (d % 4) * SF + ti * P:
                                          (d % 4) * SF + (ti + 1) * P]
                                nc.tensor.matmul(
                                    psc[ti][:],
                                    xtsl,
                                    wkva_a[:, d * (KVL + DR):
                                           d * (KVL + DR) + 320],
                                    start=(d == 0), stop=(d == NDC - 1))
                                nc.tensor.matmul(
                                    psp[ti][:],
                                    xtsl,
                                    wkva_a[:, d * (KVL + DR) + 320:
                                           (d + 1) * (KVL + DR)],
                                    start=(d == 0), stop=(d == NDC - 1))
                        for ti in range(4):
                            t = j * 4 + ti
                            sq = sqs.tile([P, KVL], F32, tag="sq", name="sq")
                            ss = nrm.tile([P, 1], F32, tag="ss", name="ss")
                            ss2 = nrm.tile([P, 1], F32, tag="ss2",
                                           name="ss2")
                            nc.scalar.activation(
                                sq[:, 0:320], psc[ti][:],
                                mybir.ActivationFunctionType.Square,
                                accum_out=ss[:])
                            nc.scalar.activation(
                                sq[:, 320:KVL], psp[ti][:, 0:192],
                                mybir.ActivationFunctionType.Square,
                                accum_out=ss2[:])
                            nc.vector.tensor_add(ss[:], ss[:], ss2[:])
                            rt_ = nrm.tile([P, 1], F32, tag="rt", name="rt")
                            nc.scalar.activation(
                                rt_[:], ss[:],
                                mybir.ActivationFunctionType.Sqrt,
                                bias=epsb[:], scale=1.0 / KVL)
                            ri = nrm.tile([P, 1], F32, tag="ri", name="ri")
                            nc.vector.reciprocal(ri[:], rt_[:])
                            nc.scalar.mul(kvc[t][:, 0:320], psc[ti][:],
                                          ri[:])
                            nc.scalar.mul(kvc[t][:, 320:KVL],
                                          psp[ti][:, 0:192], ri[:])
                            # k rope (deinterleave to [r(32) | i(32)])
                            cn = cna[:, t * 32:(t + 1) * 32]
                            sn = sna[:, t * 32:(t + 1) * 32]
                            pe = psp[ti][:, 192:256].rearrange(
                                "p (k two) -> p k two", two=2)
                            xr = pe[:, :, 0:1].rearrange(
                                "p k one -> p (k one)")
                            xi = pe[:, :, 1:2].rearrange(
                                "p k one -> p (k one)")
                            m1 = kct.tile([P, DR // 2], F32, tag="m1",
                                          name="m1")
                            m2 = kct.tile([P, DR // 2], F32, tag="m2",
                                          name="m2")
                            nc.vector.tensor_mul(m1[:], xr, cn)
                            nc.vector.tensor_mul(m2[:], xi, sn)
                            nc.vector.tensor_sub(kpe[t][:, 0:32], m1[:],
                                                 m2[:])
                            nc.vector.tensor_mul(m1[:], xr, sn)
                            nc.vector.tensor_mul(m2[:], xi, cn)
                            nc.vector.tensor_add(kpe[t][:, 32:64], m1[:],
                                                 m2[:])

            # ============ phase 2: transposes + attention ==================
            kvtp = top.enter_context(tc.tile_pool(name="kvT", bufs=NCC))
            kptp = top.enter_context(tc.tile_pool(name="kpT", bufs=1))
            msp = top.enter_context(
                tc.tile_pool(name="msp", bufs=3, space="PSUM"))
            otp = top.enter_context(tc.tile_pool(name="oT", bufs=NHC))
            oTs = [otp.tile([DV, S], RT, tag="oT", name="oT")
                   for _ in range(NHC)]
            kvcT = [kvtp.tile([P, S], RT, tag="kvcT", name="kvcT")
                    for _ in range(NCC)]
            kpeT = kptp.tile([DR, S], RT, tag="kpeT", name="kpeT")
            for t in range(NTT):
                ts_ = slice(t * P, (t + 1) * P)
                for cc in range(NCC):
                    tp = msp.tile([P, SF], F32, tag="msp", name="msp")
                    nc.tensor.transpose(
                        tp[:, 0:P],
                        kvc[t][:, cc * P:(cc + 1) * P].bitcast(F32),
                        ident[:])
                    nc.scalar.copy(kvcT[cc][:, ts_], tp[:, 0:P])
                tp = msp.tile([P, SF], F32, tag="msp", name="msp")
                nc.tensor.transpose(tp[0:DR, 0:P], kpe[t][:], ident[:])
                nc.scalar.copy(kpeT[:, ts_], tp[0:DR, 0:P])

            with ExitStack() as ph2:
                qhp = ph2.enter_context(tc.tile_pool(name="qh", bufs=2))
                qpp = ph2.enter_context(tc.tile_pool(name="qpp", bufs=2))
                wbp = ph2.enter_context(tc.tile_pool(name="wb", bufs=2))
                qap = ph2.enter_context(tc.tile_pool(name="qabs", bufs=8))
                etp = ph2.enter_context(tc.tile_pool(name="et", bufs=6))
                olp = ph2.enter_context(
                    tc.tile_pool(name="olat", bufs=4, space="PSUM"))
                dnp = ph2.enter_context(
                    tc.tile_pool(name="dn", bufs=1, space="PSUM"))
                osp = ph2.enter_context(tc.tile_pool(name="osb", bufs=8))
                dvp = ph2.enter_context(tc.tile_pool(name="dinv", bufs=2))

                for h in range(NHC):
                    qn = qhp.tile([DN, S], RT, tag="qn", name="qn")
                    nc.sync.dma_start(qn[:], d_qns.ap()[h])
                    qp = qpp.tile([DR, S], RT, tag="qp", name="qp")
                    nc.sync.dma_start(qp[:], d_qps.ap()[h])
                    wbk = wbp.tile([DN, KVL], RT, tag="wbk", name="wbk")
                    nc.sync.dma_start(wbk[:], d_wbk.ap()[h])
                    wbv = wbp.tile([P, NCC * DV], RT, tag="wbv", name="wbv")
                    nc.sync.dma_start(
                        wbv[:].rearrange("p (cc dv) -> p cc dv", cc=NCC),
                        d_wbvT.ap()[h].rearrange("(cc p) dv -> p cc dv",
                                                 p=P))
                    for j in range(NST):
                        js = slice(j * SF, (j + 1) * SF)
                        qa = [qap.tile([P, SF], RT, tag="qa", name="qa")
                              for _ in range(NCC)]
                        for cc in range(NCC):
                            ps = msp.tile([P, SF], F32, tag="msp",
                                          name="msp")
                            nc.tensor.matmul(
                                ps[:], wbk[:, cc * P:(cc + 1) * P],
                                qn[:, js], start=True, stop=True)
                            nc.vector.tensor_copy(qa[cc][:], ps[:])
                        ol = [olp.tile([P, SF], F32, tag="olat",
                                       name="olat") for _ in range(NCC)]
                        dn = dnp.tile([1, SF], F32, tag="dn", name="dn")
                        ntt = 4 * j + 4
                        for t in range(ntt):
                            ts_ = slice(t * P, (t + 1) * P)
                            # causal narrowing: diagonal tiles only need
                            # columns s >= t, i.e. local offset 128*(t-4j)
                            off = max(0, min(P * (t - 4 * j), SF - 256))
                            nf = SF - off
                            osl = slice(j * SF + off, (j + 1) * SF)
                            sc = msp.tile([P, SF], F32, tag="msp",
                                          name="msp")
                            for cc in range(NCC):
                                nc.tensor.matmul(
                                    sc[:, 0:nf], kvcT[cc][:, ts_],
                                    qa[cc][:, off:SF],
                                    start=(cc == 0), stop=False)
                            nc.tensor.matmul(
                                sc[:, 0:nf], kpeT[:, ts_], qp[:, osl],
                                start=False, stop=True)
                            e = etp.tile([P, SF], RT, tag="et", name="et")
                            nc.scalar.activation(
                                e[:, 0:nf], sc[:, 0:nf],
                                mybir.ActivationFunctionType.Exp,
                                scale=SCALE)
                            if t >= 4 * j:
                                nc.gpsimd.affine_select(
                                    out=e[:, 0:nf], in_=e[:, 0:nf],
                                    compare_op=mybir.AluOpType.is_ge,
                                    fill=0.0, base=SF * j + off - P * t,
                                    pattern=[[1, nf]],
                                    channel_multiplier=-1)
                            nc.tensor.matmul(
                                dn[:, off:SF], ones_c[:], e[:, 0:nf],
                                start=(t == 0), stop=(t == ntt - 1))
                            for cc in range(NCC):
                                nc.tensor.matmul(
                                    ol[cc][:, off:SF],
                                    kvc[t][:, cc * P:(cc + 1) * P],
                                    e[:, 0:nf], start=(t == 0),
                                    stop=(t == ntt - 1))
                        di = dvp.tile([1, SF], F32, tag="di", name="di")
                        nc.vector.reciprocal(di[:], dn[:])
                        dbp = msp.tile([P, SF], F32, tag="msp", name="msp")
                        nc.tensor.matmul(dbp[:], ones_r[:], di[:],
                                         start=True, stop=True)
                        db = dvp.tile([P, SF], F32, tag="db", name="db")
                        nc.scalar.copy(db[:], dbp[:])
                        osb = [osp.tile([P, SF], RT, tag="osb", name="osb")
                               for _ in range(NCC)]
                        for cc in range(NCC):
                            nc.scalar.copy(osb[cc][:], ol[cc][:])
                        ohps = msp.tile([P, SF], F32, tag="msp", name="msp")
                        for cc in range(NCC):
                            nc.tensor.matmul(
                                ohps[:], wbv[:, cc * DV:(cc + 1) * DV],
                                osb[cc][:],
                                start=(cc == 0), stop=(cc == NCC - 1))
                        nc.vector.tensor_mul(oTs[h][:, js], ohps[:], db[:])

            # ============ phase 3: output projection (partial) =============
            with ExitStack() as ph3:
                wop = ph3.enter_context(tc.tile_pool(name="wo", bufs=NHC))
                otg = ph3.enter_context(tc.tile_pool(name="ost", bufs=3))
                wos = [wop.tile([DV, DIM], RT, tag="wo", name="wo")
                       for _ in range(NHC)]
                for h in range(NHC):
                    nc.sync.dma_start(
                        wos[h][:], d_wo.ap()[h * DV:(h + 1) * DV, :])
                for d in range(NDC):
                    ds_ = slice(d * P, (d + 1) * P)
                    obig = otg.tile([P, S], F32, tag="ost", name="ost")
                    for j in range(NST):
                        js = slice(j * SF, (j + 1) * SF)
                        ps = msp.tile([P, SF], F32, tag="msp", name="msp")
                        for h in range(NHC):
                            nc.tensor.matmul(
                                ps[:], wos[h][:, ds_], oTs[h][:, js],
                                start=(h == 0), stop=(h == NHC - 1))
                        nc.scalar.copy(obig[:, js], ps[:])
                    nc.sync.dma_start(out[ds_, :], obig[:])

    nc.compile()
    return nc


def prep_inputs(x, wq_w, wkv_a_w, wkv_b_w, kv_norm_w, wo_w,
                freqs_cos, freqs_sin):
    """Host-side sharding/layout prep. Returns per-core input maps."""
    x = np.ascontiguousarray(np.asarray(x, np.float32).reshape(S, DIM))
    xT = np.ascontiguousarray(x.T)
    wq = np.asarray(wq_w, np.float32).reshape(DIM, NH, DN + DR)
    wkva = np.ascontiguousarray(np.asarray(wkv_a_w, np.float32))
    wkvb = np.asarray(wkv_b_w, np.float32)
    knw = np.asarray(kv_norm_w, np.float32)
    wo = np.asarray(wo_w, np.float32)
    cos = np.asarray(freqs_cos, np.float32)
    sin = np.asarray(freqs_sin, np.float32)
    cosR = np.ascontiguousarray(np.tile(cos.T, (NHC, 1)))  # [128, S]
    sinR = np.ascontiguousarray(np.tile(sin.T, (NHC, 1)))

    maps = []
    for c in range(NCORES):
        hs = list(range(NHC * c, NHC * (c + 1)))
        wq_n = np.ascontiguousarray(
            wq[:, hs, :DN].reshape(DIM, NHC * DN))
        wq_pr = np.ascontiguousarray(
            wq[:, hs, DN + 0::2].reshape(DIM, NHC * 32))
        wq_pi = np.ascontiguousarray(
            wq[:, hs, DN + 1::2].reshape(DIM, NHC * 32))
        # fold kv_norm weight into the absorbed weights
        wbk = np.stack([wkvb[h * (DN + DV):h * (DN + DV) + DN, :] * knw[None, :]
                        for h in hs])                       # [4, 128, 512]
        wbvT = np.stack(
            [np.ascontiguousarray(
                wkvb[h * (DN + DV) + DN:(h + 1) * (DN + DV), :].T)
             * knw[:, None] for h in hs])                   # [4, 512, 128]
        wo_c = np.ascontiguousarray(
            np.concatenate([wo[h * DV:(h + 1) * DV, :] for h in hs]))
        maps.append({
            "xT": xT, "wq_n": wq_n, "wq_pr": wq_pr, "wq_pi": wq_pi,
            "wkv_a": wkva, "wbk": np.ascontiguousarray(wbk),
            "wbvT": np.ascontiguousarray(wbvT), "wo_c": wo_c,
            "cos_n": cos, "sin_n": sin, "cosR": cosR, "sinR": sinR,
        })
    return maps


def kernel(x, wq_w, wkv_a_w, wkv_b_w, kv_norm_w, wo_w,
           freqs_cos, freqs_sin, start_pos):
    assert int(start_pos) == 0
    maps = prep_inputs(x, wq_w, wkv_a_w, wkv_b_w, kv_norm_w, wo_w,
                       freqs_cos, freqs_sin)
    nc = build_nc()
    res = run_bass_kernel_spmd(nc, maps, list(range(NCORES)))
    acc = np.zeros((DIM, S), np.float64)
    for c in range(NCORES):
        acc += res.results[c]["outT"]
    return np.ascontiguousarray(acc.T).astype(np.float32).reshape(1, S, DIM)



# revision 5
# speedup vs baseline: 1.6931x; 1.6931x over previous
"""MLA (multi-head latent attention) prefill block on 8 Trainium2 NeuronCores.

Tensor-parallel over heads: each core computes 4 of the 32 heads end-to-end.
Unlike the absorbed (decode-style) formulation, this kernel materializes
per-head K = kv_c @ wbk^T [S, 128] and V = kv_c @ wbv [S, 128] explicitly,
so scores contract over 192 dims (128 nope + 64 rope) instead of 576 and
the attention output contracts over 128 instead of 512 — ~1.6x fewer MACs.

All matmul operands are bf16 (1 cycle/row on the PE, same as f32r, but half
the SBUF/DMA traffic); softmax statistics and rope stay f32. Per-core
partial outputs of the row-parallel wo matmul are summed on the host.

Self-contained: hardcodes all shapes from the problem spec.
"""

import os
from contextlib import ExitStack

import numpy as np

import concourse.bacc as bacc
import concourse.bass as bass
import concourse.mybir as mybir
import concourse.tile as tile
from concourse.bass_utils import run_bass_kernel_spmd
from concourse.masks import make_identity

# ---- problem constants ----
DIM = 2048
NH = 32
DN = 128  # qk_nope_head_dim
DR = 64   # qk_rope_head_dim
DV = 128  # v_head_dim
KVL = 512  # kv_lora_rank
S = 2048   # sequence length (B=1)
SCALE = float((DN + DR) ** -0.5)
EPS = 1e-6

NCORES = 8
NHC = NH // NCORES      # heads per core = 4
P = 128                 # partitions
SF = 512                # free-dim tile (s tiles)
NST = S // SF           # 4 s tiles
NTT = S // P            # 16 t tiles
NDC = DIM // P          # 16 contraction chunks over model dim
NCC = KVL // P          # 4 latent chunks

F32 = mybir.dt.float32
BF16 = mybir.dt.bfloat16
F16 = mybir.dt.float16
RT = BF16  # dtype for all matmul operands


def build_nc(repeat=1):
    """Build the per-core Bass program (identical on all 8 cores)."""
    nc = bacc.Bacc("TRN2", target_bir_lowering=False, debug=False,
                   num_devices=NCORES)

    # ---- DRAM I/O ----
    d_xT = nc.dram_tensor("xT", [DIM, S], RT, kind="ExternalInput")
    d_wqn = nc.dram_tensor("wq_n", [DIM, NHC * DN], RT, kind="ExternalInput")
    d_wqpr = nc.dram_tensor("wq_pr", [DIM, NHC * 32], RT, kind="ExternalInput")
    d_wqpi = nc.dram_tensor("wq_pi", [DIM, NHC * 32], RT, kind="ExternalInput")
    d_wkva = nc.dram_tensor("wkv_a", [DIM, KVL + DR], RT, kind="ExternalInput")
    d_wbkT = nc.dram_tensor("wbkT", [NHC, KVL, DN], RT, kind="ExternalInput")
    d_wbv = nc.dram_tensor("wbv_all", [KVL, NHC * DV], RT,
                           kind="ExternalInput")
    d_wo = nc.dram_tensor("wo_c", [NHC * DV, DIM], RT, kind="ExternalInput")
    d_cosn = nc.dram_tensor("cos_n", [S, DR // 2], F32, kind="ExternalInput")
    d_sinn = nc.dram_tensor("sin_n", [S, DR // 2], F32, kind="ExternalInput")
    d_cosr = nc.dram_tensor("cosR", [P, S], F32, kind="ExternalInput")
    d_sinr = nc.dram_tensor("sinR", [P, S], F32, kind="ExternalInput")
    d_out = nc.dram_tensor("outT", [DIM, S], F16, kind="ExternalOutput")

    out = d_out.ap()

    with tile.TileContext(nc) as tc:
      for _rep in range(repeat):
        with ExitStack() as top:
            cst = top.enter_context(tc.tile_pool(name="const", bufs=1))
            ident = cst.tile([P, P], RT, tag="ident", name="ident")
            make_identity(nc, ident[:])
            ones_c = cst.tile([P, 1], RT, tag="ones_c", name="ones_c")
            nc.gpsimd.memset(ones_c[:], 1.0)
            ones_r = cst.tile([1, P], RT, tag="ones_r", name="ones_r")
            nc.gpsimd.memset(ones_r[:], 1.0)
            epsb = cst.tile([P, 1], F32, tag="epsb", name="epsb")
            nc.gpsimd.memset(epsb[:], EPS)

            # long-lived: normalized latent kv (natural layout) + k rope
            kvp = top.enter_context(tc.tile_pool(name="kv", bufs=NTT))
            kvc = [kvp.tile([P, KVL], RT, tag="kvc", name="kvc")
                   for _ in range(NTT)]
            kpp = top.enter_context(tc.tile_pool(name="kpe", bufs=NTT))
            kpe = [kpp.tile([P, DR], RT, tag="kpe", name="kpe")
                   for _ in range(NTT)]
            # q for all 4 heads, kept in SBUF
            qnp = top.enter_context(tc.tile_pool(name="qn", bufs=NHC))
            qns = [qnp.tile([DN, S], RT, tag="qn", name="qn")
                   for _ in range(NHC)]
            qpp = top.enter_context(tc.tile_pool(name="qp", bufs=NHC))
            qps = [qpp.tile([DR, S], RT, tag="qp", name="qp")
                   for _ in range(NHC)]

            # ===== phase 1: q + kv projections, single pass over xT ========
            with ExitStack() as ph1:
                wrp = ph1.enter_context(tc.tile_pool(name="wres", bufs=1))
                xsl = ph1.enter_context(tc.tile_pool(name="xsl", bufs=6))
                xTj0 = d_xT.ap()[:, 0:SF].rearrange("(d p) f -> p d f", p=P)
                xh0 = [xsl.tile([P, 4 * SF], RT, tag="xsl", name="xsl")
                       for _ in range(4)]
                wqn_a = wrp.tile([P, NDC * NHC * DN], RT, tag="wqn",
                                 name="wqn")
                wqpr_a = wrp.tile([P, NDC * NHC * 32], RT, tag="wqpr",
                                  name="wqpr")
                wqpi_a = wrp.tile([P, NDC * NHC * 32], RT, tag="wqpi",
                                  name="wqpi")
                wkva_a = wrp.tile([P, NDC * (KVL + DR)], RT, tag="wkva",
                                  name="wkva")
                # interleave x and weight quarters in consumption order so
                # the PE starts after ~2 quarters instead of the full set
                for q4 in range(4):
                    hd = slice(q4 * (NDC // 4), (q4 + 1) * (NDC // 4))
                    nc.sync.dma_start(
                        xh0[q4][:].rearrange("p (d f) -> p d f", d=4),
                        xTj0[:, 4 * q4:4 * (q4 + 1)])
                    nc.sync.dma_start(
                        wqn_a[:].rearrange("p (d c) -> p d c", d=NDC)[:, hd],
                        d_wqn.ap().rearrange("(d p) c -> p d c", p=P)[:, hd])
                    nc.sync.dma_start(
                        wqpr_a[:].rearrange("p (d c) -> p d c", d=NDC)[:, hd],
                        d_wqpr.ap().rearrange("(d p) c -> p d c", p=P)[:, hd])
                    nc.sync.dma_start(
                        wqpi_a[:].rearrange("p (d c) -> p d c", d=NDC)[:, hd],
                        d_wqpi.ap().rearrange("(d p) c -> p d c", p=P)[:, hd])
                    nc.sync.dma_start(
                        wkva_a[:].rearrange("p (d c) -> p d c", d=NDC)[:, hd],
                        d_wkva.ap().rearrange("(d p) c -> p d c", p=P)[:, hd])
                cna = wrp.tile([P, NTT * 32], F32, tag="cna", name="cna")
                sna = wrp.tile([P, NTT * 32], F32, tag="sna", name="sna")
                nc.sync.dma_start(
                    cna[:].rearrange("p (t k) -> p t k", t=NTT),
                    d_cosn.ap().rearrange("(t p) k -> p t k", p=P))
                nc.sync.dma_start(
                    sna[:].rearrange("p (t k) -> p t k", t=NTT),
                    d_sinn.ap().rearrange("(t p) k -> p t k", p=P))

                rts = ph1.enter_context(tc.tile_pool(name="ropetmp", bufs=1))
                sqs = ph1.enter_context(tc.tile_pool(name="sqs", bufs=2))
                crs = ph1.enter_context(tc.tile_pool(name="crs", bufs=2))
                kct = ph1.enter_context(tc.tile_pool(name="kct", bufs=2))
                nrm = ph1.enter_context(tc.tile_pool(name="nrm", bufs=4))

                with tc.tile_pool(name="acc1", bufs=8, space="PSUM") as qac:
                    for j in range(NST):
                        js = slice(j * SF, (j + 1) * SF)
                        xTj = d_xT.ap()[:, js].rearrange(
                            "(d p) f -> p d f", p=P)
                        if j == 0:
                            xh = xh0
                        else:
                            xh = [xsl.tile([P, 4 * SF], RT, tag="xsl",
                                           name="xsl") for _ in range(4)]
                            for q4 in range(4):
                                nc.sync.dma_start(
                                    xh[q4][:].rearrange(
                                        "p (d f) -> p d f", d=4),
                                    xTj[:, 4 * q4:4 * (q4 + 1)])
                        # ---- q projections for this s block ----
                        pss = [qac.tile([P, SF], F32, tag="acc", name="acc")
                               for _ in range(NHC + 2)]
                        for d in range(NDC):
                            xs = xh[d // 4][:, (d % 4) * SF:(d % 4 + 1) * SF]
                            for h in range(NHC):
                                nc.tensor.matmul(
                                    pss[h][:],
                                    wqn_a[:, d * NHC * DN + h * DN:
                                          d * NHC * DN + (h + 1) * DN],
                                    xs,
                                    start=(d == 0), stop=(d == NDC - 1))
                            nc.tensor.matmul(
                                pss[NHC][:],
                                wqpr_a[:, d * P:(d + 1) * P], xs,
                                start=(d == 0), stop=(d == NDC - 1))
                            nc.tensor.matmul(
                                pss[NHC + 1][:],
                                wqpi_a[:, d * P:(d + 1) * P], xs,
                                start=(d == 0), stop=(d == NDC - 1))
                        for h in range(NHC):
                            nc.scalar.copy(qns[h][:, js], pss[h][:])
                        # rope rotation for q_pe (even=r, odd=i) off PSUM
                        t1 = rts.tile([P, SF], F32, tag="t1", name="t1")
                        t2 = rts.tile([P, SF], F32, tag="t2", name="t2")
                        ror = rts.tile([P, SF], F32, tag="ror", name="ror")
                        roi = rts.tile([P, SF], F32, tag="roi", name="roi")
                        cR = crs.tile([P, SF], F32, tag="cR", name="cR")
                        sR = crs.tile([P, SF], F32, tag="sR", name="sR")
                        nc.sync.dma_start(cR[:], d_cosr.ap()[:, js])
                        nc.sync.dma_start(sR[:], d_sinr.ap()[:, js])
                        nc.vector.tensor_mul(t1[:], pss[NHC][:], cR[:])
                        nc.vector.tensor_mul(t2[:], pss[NHC + 1][:], sR[:])
                        nc.vector.tensor_sub(ror[:], t1[:], t2[:])
                        nc.vector.tensor_mul(t1[:], pss[NHC][:], sR[:])
                        nc.vector.tensor_mul(t2[:], pss[NHC + 1][:], cR[:])
                        nc.vector.tensor_add(roi[:], t1[:], t2[:])
                        for h in range(NHC):
                            hs = slice(h * 32, (h + 1) * 32)
                            nc.vector.tensor_copy(
                                qps[h][0:32, js], ror[hs, :])
                            nc.vector.tensor_copy(
                                qps[h][32:64, js], roi[hs, :])
                        # ---- kv projection for this t block (same x) ----
                        for ti in range(4):
                            psc = qac.tile([P, KVL], F32, tag="acc",
                                           name="acc")
                            psp = qac.tile([P, DR], F32, tag="acc",
                                           name="accp", padded_shape=[P, SF])
                            for d in range(NDC):
                                xs = xh[d // 4]
                                xtsl = xs[:, (d % 4) * SF + ti * P:
                                          (d % 4) * SF + (ti + 1) * P]
                                nc.tensor.matmul(
                                    psc[:], xtsl,
                                    wkva_a[:, d * (KVL + DR):
                                           d * (KVL + DR) + KVL],
                                    start=(d == 0), stop=(d == NDC - 1))
                                nc.tensor.matmul(
                                    psp[:], xtsl,
                                    wkva_a[:, d * (KVL + DR) + KVL:
                                           (d + 1) * (KVL + DR)],
                                    start=(d == 0), stop=(d == NDC - 1))
                            t = j * 4 + ti
                            sq = sqs.tile([P, KVL], F32, tag="sq", name="sq")
                            ss = nrm.tile([P, 1], F32, tag="ss", name="ss")
                            nc.scalar.activation(
                                sq[:], psc[:],
                                mybir.ActivationFunctionType.Square,
                                accum_out=ss[:])
                            rt_ = nrm.tile([P, 1], F32, tag="rt", name="rt")
                            nc.scalar.activation(
                                rt_[:], ss[:],
                                mybir.ActivationFunctionType.Sqrt,
                                bias=epsb[:], scale=1.0 / KVL)
                            ri = nrm.tile([P, 1], F32, tag="ri", name="ri")
                            nc.vector.reciprocal(ri[:], rt_[:])
                            nc.scalar.mul(kvc[t][:], psc[:], ri[:])
                            # k rope (deinterleave to [r(32) | i(32)])
                            cn = cna[:, t * 32:(t + 1) * 32]
                            sn = sna[:, t * 32:(t + 1) * 32]
                            pe = psp[:].rearrange(
                                "p (k two) -> p k two", two=2)
                            xr = pe[:, :, 0:1].rearrange(
                                "p k one -> p (k one)")
                            xi = pe[:, :, 1:2].rearrange(
                                "p k one -> p (k one)")
                            m1 = kct.tile([P, DR // 2], F32, tag="m1",
                                          name="m1")
                            m2 = kct.tile([P, DR // 2], F32, tag="m2",
                                          name="m2")
                            nc.vector.tensor_mul(m1[:], xr, cn)
                            nc.vector.tensor_mul(m2[:], xi, sn)
                            nc.vector.tensor_sub(kpe[t][:, 0:32], m1[:],
                                                 m2[:])
                            nc.vector.tensor_mul(m1[:], xr, sn)
                            nc.vector.tensor_mul(m2[:], xi, cn)
                            nc.vector.tensor_add(kpe[t][:, 32:64], m1[:],
                                                 m2[:])

            # ======== phase 2: transposes, K/V materialize, attention ======
            kvtp = top.enter_context(tc.tile_pool(name="kvT", bufs=NCC))
            kptp = top.enter_context(tc.tile_pool(name="kpT", bufs=1))
            msp = top.enter_context(
                tc.tile_pool(name="msp", bufs=3, space="PSUM"))
            otp = top.enter_context(tc.tile_pool(name="oT", bufs=NHC))
            oTs = [otp.tile([DV, S], RT, tag="oT", name="oT")
                   for _ in range(NHC)]
            kvcT = [kvtp.tile([P, S], RT, tag="kvcT", name="kvcT")
                    for _ in range(NCC)]
            kpeT = kptp.tile([DR, S], RT, tag="kpeT", name="kpeT")
            with tc.tile_pool(name="tpp", bufs=4, space="PSUM") as tpp:
                for t in range(NTT):
                    ts_ = slice(t * P, (t + 1) * P)
                    for cc in range(NCC):
                        tp = tpp.tile([P, P], RT, tag="mspt", name="mspt")
                        nc.tensor.transpose(
                            tp[:], kvc[t][:, cc * P:(cc + 1) * P], ident[:])
                        nc.scalar.copy(kvcT[cc][:, ts_], tp[:])
                    tp = tpp.tile([P, P], RT, tag="mspt", name="mspt")
                    nc.tensor.transpose(tp[0:DR, :], kpe[t][:], ident[:])
                    nc.scalar.copy(kpeT[:, ts_], tp[0:DR, :])

            # V for all 4 heads: V_all[t] = kv_c[t] @ wbv  -> [t(128), 4*DV]
            vap = top.enter_context(tc.tile_pool(name="vall", bufs=NTT))
            wbvp = top.enter_context(tc.tile_pool(name="wbv", bufs=1))
            wbv_a = wbvp.tile([P, NCC * NHC * DV], RT, tag="wbv",
                              name="wbv")
            nc.sync.dma_start(
                wbv_a[:].rearrange("p (cc f) -> p cc f", cc=NCC),
                d_wbv.ap().rearrange("(cc p) f -> p cc f", p=P))
            vall = [vap.tile([P, NHC * DV], RT, tag="vall", name="vall")
                    for _ in range(NTT)]
            for t in range(NTT):
                ts_ = slice(t * P, (t + 1) * P)
                ps = msp.tile([P, SF], F32, tag="msp", name="msp")
                for cc in range(NCC):
                    nc.tensor.matmul(
                        ps[:], kvcT[cc][:, ts_],
                        wbv_a[:, cc * NHC * DV:(cc + 1) * NHC * DV],
                        start=(cc == 0), stop=(cc == NCC - 1))
                nc.scalar.copy(vall[t][:], ps[:])

            with ExitStack() as ph2:
                wbkp = ph2.enter_context(tc.tile_pool(name="wbk", bufs=2))
                ktp = ph2.enter_context(tc.tile_pool(name="kt", bufs=2))
                etp = ph2.enter_context(tc.tile_pool(name="et", bufs=6))
                ohp = ph2.enter_context(
                    tc.tile_pool(name="ohp", bufs=2, space="PSUM"))
                dnp = ph2.enter_context(
                    tc.tile_pool(name="dn", bufs=2, space="PSUM"))
                dvp = ph2.enter_context(tc.tile_pool(name="dinv", bufs=2))

                for h in range(NHC):
                    # K_h^T = wbk_h^T(scaled) @ kv_c^T   [DN, S]
                    wbk = wbkp.tile([P, NCC * DN], RT, tag="wbk", name="wbk")
                    nc.sync.dma_start(
                        wbk[:].rearrange("p (cc f) -> p cc f", cc=NCC),
                        d_wbkT.ap()[h].rearrange("(cc p) f -> p cc f", p=P))
                    kT = ktp.tile([DN, S], RT, tag="kT", name="kT")
                    for j in range(NST):
                        js = slice(j * SF, (j + 1) * SF)
                        ps = msp.tile([P, SF], F32, tag="msp", name="msp")
                        for cc in range(NCC):
                            nc.tensor.matmul(
                                ps[:], wbk[:, cc * DN:(cc + 1) * DN],
                                kvcT[cc][:, js],
                                start=(cc == 0), stop=(cc == NCC - 1))
                        nc.scalar.copy(kT[:, js], ps[:])
                    for j in range(NST):
                        js = slice(j * SF, (j + 1) * SF)
                        oh = ohp.tile([P, SF], F32, tag="oh", name="oh")
                        dn = dnp.tile([1, SF], F32, tag="dn", name="dn")
                        ntt = 4 * j + 4
                        for t in range(ntt):
                            ts_ = slice(t * P, (t + 1) * P)
                            # causal narrowing: tile t only needs columns
                            # s >= t*128, i.e. local offset 128*(t-4j)
                            off = max(0, P * (t - 4 * j))
                            nf = SF - off
                            osl = slice(j * SF + off, (j + 1) * SF)
                            sc = msp.tile([P, SF], F32, tag="msp",
                                          name="msp")
                            nc.tensor.matmul(
                                sc[:, 0:nf], kT[:, ts_], qns[h][:, osl],
                                start=True, stop=False)
                            nc.tensor.matmul(
                                sc[:, 0:nf], kpeT[:, ts_], qps[h][:, osl],
                                start=False, stop=True)
                            e = etp.tile([P, SF], RT, tag="et", name="et")
                            nc.scalar.activation(
                                e[:, 0:nf], sc[:, 0:nf],
                                mybir.ActivationFunctionType.Exp,
                                scale=SCALE)
                            if t >= 4 * j:
                                # triangular mask on the first 128 columns
                                # of the diagonal tile (keep c >= p)
                                nc.gpsimd.affine_select(
                                    out=e[:, 0:P], in_=e[:, 0:P],
                                    compare_op=mybir.AluOpType.is_ge,
                                    fill=0.0, base=0,
                                    pattern=[[1, P]],
                                    channel_multiplier=-1)
                            nc.tensor.matmul(
                                dn[:, off:SF], ones_c[:], e[:, 0:nf],
                                start=(t == 0), stop=(t == ntt - 1))
                            nc.tensor.matmul(
                                oh[:, off:SF],
                                vall[t][:, h * DV:(h + 1) * DV],
                                e[:, 0:nf], start=(t == 0),
                                stop=(t == ntt - 1))
                        di = dvp.tile([1, SF], RT, tag="di", name="di")
                        with nc.allow_low_precision(
                                reason="bf16 1/denom is within tolerance"):
                            nc.vector.reciprocal(di[:], dn[:])
                        dbp = msp.tile([P, SF], F32, tag="msp", name="msp")
                        nc.tensor.matmul(dbp[:], ones_r[:], di[:],
                                         start=True, stop=True)
                        db = dvp.tile([P, SF], F32, tag="db", name="db")
                        nc.scalar.copy(db[:], dbp[:])
                        nc.vector.tensor_mul(oTs[h][:, js], oh[:], db[:])

            # ============ phase 3: output projection (partial) =============
            with ExitStack() as ph3:
                wop = ph3.enter_context(tc.tile_pool(name="wo", bufs=NHC))
                otg = ph3.enter_context(tc.tile_pool(name="ost", bufs=3))
                wos = [wop.tile([DV, DIM], RT, tag="wo", name="wo")
                       for _ in range(NHC)]
                for h in range(NHC):
                    nc.sync.dma_start(
                        wos[h][:], d_wo.ap()[h * DV:(h + 1) * DV, :])
                for d in range(NDC):
                    ds_ = slice(d * P, (d + 1) * P)
                    obig = otg.tile([P, S], F16, tag="ost", name="ost")
                    for j in range(NST):
                        js = slice(j * SF, (j + 1) * SF)
                        ps = msp.tile([P, SF], F32, tag="msp", name="msp")
                        for h in range(NHC):
                            nc.tensor.matmul(
                                ps[:], wos[h][:, ds_], oTs[h][:, js],
                                start=(h == 0), stop=(h == NHC - 1))
                        nc.scalar.copy(obig[:, js], ps[:])
                    nc.sync.dma_start(out[ds_, :], obig[:])

    nc.compile()
    return nc


def prep_inputs(x, wq_w, wkv_a_w, wkv_b_w, kv_norm_w, wo_w,
                freqs_cos, freqs_sin):
    """Host-side sharding/layout prep. Returns per-core input maps."""
    import ml_dtypes
    bf16 = ml_dtypes.bfloat16
    x = np.ascontiguousarray(np.asarray(x, np.float32).reshape(S, DIM))
    xT = np.ascontiguousarray(x.T).astype(bf16)
    wq = np.asarray(wq_w, np.float32).reshape(DIM, NH, DN + DR)
    wkva = np.ascontiguousarray(np.asarray(wkv_a_w, np.float32)).astype(bf16)
    wkvb = np.asarray(wkv_b_w, np.float32)
    knw = np.asarray(kv_norm_w, np.float32)
    wo = np.asarray(wo_w, np.float32)
    cos = np.asarray(freqs_cos, np.float32)
    sin = np.asarray(freqs_sin, np.float32)
    cosR = np.ascontiguousarray(np.tile(cos.T, (NHC, 1)))  # [128, S]
    sinR = np.ascontiguousarray(np.tile(sin.T, (NHC, 1)))

    maps = []
    for c in range(NCORES):
        hs = list(range(NHC * c, NHC * (c + 1)))
        wq_n = np.ascontiguousarray(
            wq[:, hs, :DN].reshape(DIM, NHC * DN)).astype(bf16)
        wq_pr = np.ascontiguousarray(
            wq[:, hs, DN + 0::2].reshape(DIM, NHC * 32)).astype(bf16)
        wq_pi = np.ascontiguousarray(
            wq[:, hs, DN + 1::2].reshape(DIM, NHC * 32)).astype(bf16)
        # fold kv_norm weight into the absorbed weights
        # wbkT[h] = (wbk_h * knw).T  [KVL, DN]
        wbkT = np.stack([
            np.ascontiguousarray(
                (wkvb[h * (DN + DV):h * (DN + DV) + DN, :] * knw[None, :]).T)
            for h in hs]).astype(bf16)                      # [4, 512, 128]
        # wbv_all = concat_h (wbv_h^T * knw[:,None])  [KVL, 4*DV]
        wbv_all = np.concatenate(
            [np.ascontiguousarray(
                wkvb[h * (DN + DV) + DN:(h + 1) * (DN + DV), :].T)
             * knw[:, None] for h in hs], axis=1).astype(bf16)  # [512, 512]
        wo_c = np.ascontiguousarray(
            np.concatenate([wo[h * DV:(h + 1) * DV, :]
                            for h in hs])).astype(bf16)
        maps.append({
            "xT": xT, "wq_n": wq_n, "wq_pr": wq_pr, "wq_pi": wq_pi,
            "wkv_a": wkva, "wbkT": np.ascontiguousarray(wbkT),
            "wbv_all": np.ascontiguousarray(wbv_all), "wo_c": wo_c,
            "cos_n": cos, "sin_n": sin, "cosR": cosR, "sinR": sinR,
        })
    return maps


def kernel(x, wq_w, wkv_a_w, wkv_b_w, kv_norm_w, wo_w,
           freqs_cos, freqs_sin, start_pos):
    assert int(start_pos) == 0
    maps = prep_inputs(x, wq_w, wkv_a_w, wkv_b_w, kv_norm_w, wo_w,
                       freqs_cos, freqs_sin)
    nc = build_nc()
    res = run_bass_kernel_spmd(nc, maps, list(range(NCORES)))
    acc = np.zeros((DIM, S), np.float64)
    for c in range(NCORES):
        acc += res.results[c]["outT"].astype(np.float64)
    return np.ascontiguousarray(acc.T).astype(np.float32).reshape(1, S, DIM)


# revision 13
# speedup vs baseline: 3.1358x; 1.8521x over previous
"""MLA (multi-head latent attention) prefill block on 8 Trainium2 NeuronCores.

Tensor-parallel over heads: each core computes 4 of the 32 heads end-to-end.
Unlike the absorbed (decode-style) formulation, this kernel materializes
per-head K = kv_c @ wbk^T [S, 128] and V = kv_c @ wbv [S, 128] explicitly,
so scores contract over 192 dims (128 nope + 64 rope) instead of 576 and
the attention output contracts over 128 instead of 512 — ~1.6x fewer MACs.

The kv_a projection + rms-norm + rope (otherwise replicated on all 8 cores)
is sharded over the sequence: each core computes 2 of the 16 kv tiles,
transposes them, and an AllGather collective distributes the transposed
latents while the PE runs the q projections.

All matmul operands are bf16 (1 cycle/row on the PE, same as f32r, but half
the SBUF/DMA traffic); softmax statistics and rope stay f32. Per-core
partial outputs of the row-parallel wo matmul are summed on the host.

Self-contained: hardcodes all shapes from the problem spec.
"""

import os
from contextlib import ExitStack

import numpy as np

import concourse.bacc as bacc
import concourse.bass as bass
import concourse.mybir as mybir
import concourse.tile as tile
from concourse.bass_utils import run_bass_kernel_spmd
from concourse.masks import make_identity

# ---- problem constants ----
DIM = 2048
NH = 32
DN = 128  # qk_nope_head_dim
DR = 64   # qk_rope_head_dim
DV = 128  # v_head_dim
KVL = 512  # kv_lora_rank
S = 2048   # sequence length (B=1)
SCALE = float((DN + DR) ** -0.5)
EPS = 1e-6

NCORES = 8
NHC = NH // NCORES      # heads per core = 4
P = 128                 # partitions
SF = 512                # free-dim tile (s tiles)
NST = S // SF           # 4 s tiles
NTT = S // P            # 16 t tiles
NDC = DIM // P          # 16 contraction chunks over model dim
NCC = KVL // P          # 4 latent chunks

F32 = mybir.dt.float32
BF16 = mybir.dt.bfloat16
F16 = mybir.dt.float16
RT = BF16  # dtype for all matmul operands

# Shard the kv_a projection across cores + AllGather (vs replicate)
SHARD_KV = os.environ.get("MLA_SHARD_KV", "1") == "1"
TSH = S // NCORES  # 256 seq positions (2 t tiles) owned per core


def build_nc(repeat=1):
    """Build the per-core Bass program (identical on all 8 cores)."""
    nc = bacc.Bacc("TRN2", target_bir_lowering=False, debug=False,
                   num_devices=NCORES)

    # ---- DRAM I/O ----
    d_xT = nc.dram_tensor("xT", [DIM, S], RT, kind="ExternalInput")
    d_wqn = nc.dram_tensor("wq_n", [DIM, NHC * DN], RT, kind="ExternalInput")
    d_wqpr = nc.dram_tensor("wq_pr", [DIM, NHC * 32], RT, kind="ExternalInput")
    d_wqpi = nc.dram_tensor("wq_pi", [DIM, NHC * 32], RT, kind="ExternalInput")
    d_wkva = nc.dram_tensor("wkv_a", [DIM, KVL + DR], RT, kind="ExternalInput")
    d_wbkT = nc.dram_tensor("wbkT", [NHC, KVL, DN], RT, kind="ExternalInput")
    d_wbv = nc.dram_tensor("wbv_all", [KVL, NHC * DV], RT,
                           kind="ExternalInput")
    d_wo = nc.dram_tensor("wo_c", [NHC * DV, DIM], RT, kind="ExternalInput")
    d_cosr = nc.dram_tensor("cosR", [P, S], F32, kind="ExternalInput")
    d_sinr = nc.dram_tensor("sinR", [P, S], F32, kind="ExternalInput")
    if SHARD_KV:
        d_xo = nc.dram_tensor("x_own", [DIM, TSH], RT, kind="ExternalInput")
        d_coso = nc.dram_tensor("cos_o", [P, 2 * 32], F32,
                                kind="ExternalInput")
        d_sino = nc.dram_tensor("sin_o", [P, 2 * 32], F32,
                                kind="ExternalInput")
    else:
        d_cosn = nc.dram_tensor("cos_n", [S, DR // 2], F32,
                                kind="ExternalInput")
        d_sinn = nc.dram_tensor("sin_n", [S, DR // 2], F32,
                                kind="ExternalInput")
    d_out = nc.dram_tensor("outT", [DIM, S], F16, kind="ExternalOutput")

    out = d_out.ap()

    with tile.TileContext(nc) as tc:
      for _rep in range(repeat):
        with ExitStack() as top:
            cst = top.enter_context(tc.tile_pool(name="const", bufs=1))
            ident = cst.tile([P, P], RT, tag="ident", name="ident")
            make_identity(nc, ident[:])
            ones_c = cst.tile([P, 1], RT, tag="ones_c", name="ones_c")
            nc.gpsimd.memset(ones_c[:], 1.0)
            epsb = cst.tile([P, 1], F32, tag="epsb", name="epsb")
            nc.gpsimd.memset(epsb[:], EPS)

            # transposed latents, shared by all heads
            kvtp = top.enter_context(tc.tile_pool(name="kvT", bufs=NCC))
            kptp = top.enter_context(tc.tile_pool(name="kpT", bufs=1))
            kvcT = [kvtp.tile([P, S], RT, tag="kvcT", name="kvcT")
                    for _ in range(NCC)]
            kpeT = kptp.tile([DR, S], RT, tag="kpeT", name="kpeT")
            # q for all 4 heads, kept in SBUF
            qnp = top.enter_context(tc.tile_pool(name="qn", bufs=NHC))
            qns = [qnp.tile([DN, S], RT, tag="qn", name="qn")
                   for _ in range(NHC)]
            qpp = top.enter_context(tc.tile_pool(name="qp", bufs=NHC))
            qps = [qpp.tile([DR, S], RT, tag="qp", name="qp")
                   for _ in range(NHC)]
            wkp = top.enter_context(tc.tile_pool(name="wkva", bufs=1))
            wkva_a = wkp.tile([P, NDC * (KVL + DR)], RT, tag="wkva",
                              name="wkva")

            # ===== phase 0: kv shard (2 t-tiles) + AllGather ===============
            if SHARD_KV:
              with ExitStack() as ph0:
                p0s = ph0.enter_context(tc.tile_pool(name="p0s", bufs=1))
                xo = p0s.tile([P, NDC * TSH], RT, tag="xo", name="xo")
                # interleave x-shard and wkva quarters in consumption order
                for q4 in range(4):
                    hd = slice(q4 * (NDC // 4), (q4 + 1) * (NDC // 4))
                    nc.sync.dma_start(
                        xo[:].rearrange("p (d f) -> p d f", d=NDC)[:, hd],
                        d_xo.ap().rearrange("(d p) f -> p d f", p=P)[:, hd])
                    nc.sync.dma_start(
                        wkva_a[:].rearrange("p (d c) -> p d c",
                                            d=NDC)[:, hd],
                        d_wkva.ap().rearrange("(d p) c -> p d c",
                                              p=P)[:, hd])
                coso = p0s.tile([P, 2 * 32], F32, tag="coso", name="coso")
                sino = p0s.tile([P, 2 * 32], F32, tag="sino", name="sino")
                nc.sync.dma_start(coso[:], d_coso.ap())
                nc.sync.dma_start(sino[:], d_sino.ap())
                shT = p0s.tile([P, 5 * TSH], RT, tag="shT", name="shT")
                kvo = [p0s.tile([P, KVL], RT, tag="kvo", name="kvo")
                       for _ in range(2)]
                kpo = [p0s.tile([P, DR], RT, tag="kpo", name="kpo")
                       for _ in range(2)]
                nrm0 = ph0.enter_context(tc.tile_pool(name="nrm0", bufs=2))
                with tc.tile_pool(name="p0a", bufs=4, space="PSUM") as p0a:
                    for ti in range(2):
                        psc = p0a.tile([P, KVL], F32, tag="acc", name="acc")
                        psp = p0a.tile([P, DR], F32, tag="acc",
                                       name="accp", padded_shape=[P, KVL])
                        for d in range(NDC):
                            xtsl = xo[:, d * TSH + ti * P:
                                      d * TSH + (ti + 1) * P]
                            nc.tensor.matmul(
                                psc[:], xtsl,
                                wkva_a[:, d * (KVL + DR):
                                       d * (KVL + DR) + KVL],
                                start=(d == 0), stop=(d == NDC - 1))
                            nc.tensor.matmul(
                                psp[:], xtsl,
                                wkva_a[:, d * (KVL + DR) + KVL:
                                       (d + 1) * (KVL + DR)],
                                start=(d == 0), stop=(d == NDC - 1))
                        sq = nrm0.tile([P, KVL], F32, tag="sq", name="sq")
                        ss = nrm0.tile([P, 1], F32, tag="ss", name="ss")
                        nc.scalar.activation(
                            sq[:], psc[:],
                            mybir.ActivationFunctionType.Square,
                            accum_out=ss[:])
                        rt_ = nrm0.tile([P, 1], F32, tag="rt", name="rt")
                        nc.scalar.activation(
                            rt_[:], ss[:],
                            mybir.ActivationFunctionType.Sqrt,
                            bias=epsb[:], scale=1.0 / KVL)
                        ri = nrm0.tile([P, 1], F32, tag="ri", name="ri")
                        nc.vector.reciprocal(ri[:], rt_[:])
                        nc.scalar.mul(kvo[ti][:], psc[:], ri[:])
                        # k rope (deinterleave to [r(32) | i(32)])
                        cn = coso[:, ti * 32:(ti + 1) * 32]
                        sn = sino[:, ti * 32:(ti + 1) * 32]
                        pe = psp[:].rearrange("p (k two) -> p k two", two=2)
                        xr = pe[:, :, 0:1].rearrange("p k one -> p (k one)")
                        xi = pe[:, :, 1:2].rearrange("p k one -> p (k one)")
                        m1 = nrm0.tile([P, DR // 2], F32, tag="m1", name="m1")
                        m2 = nrm0.tile([P, DR // 2], F32, tag="m2", name="m2")
                        nc.vector.tensor_mul(m1[:], xr, cn)
                        nc.vector.tensor_mul(m2[:], xi, sn)
                        nc.vector.tensor_sub(kpo[ti][:, 0:32], m1[:], m2[:])
                        nc.vector.tensor_mul(m1[:], xr, sn)
                        nc.vector.tensor_mul(m2[:], xi, cn)
                        nc.vector.tensor_add(kpo[ti][:, 32:64], m1[:], m2[:])
                    # transpose own tiles into shard layout
                    with tc.tile_pool(name="tp0", bufs=4,
                                      space="PSUM") as tp0:
                        for ti in range(2):
                            for cc in range(NCC):
                                tp = tp0.tile([P, P], RT, tag="t", name="t")
                                nc.tensor.transpose(
                                    tp[:],
                                    kvo[ti][:, cc * P:(cc + 1) * P],
                                    ident[:])
                                nc.scalar.copy(
                                    shT[:, cc * TSH + ti * P:
                                        cc * TSH + (ti + 1) * P], tp[:])
                            tp = tp0.tile([P, P], RT, tag="t", name="t")
                            nc.tensor.transpose(tp[0:DR, :], kpo[ti][:],
                                                ident[:])
                            nc.scalar.copy(
                                shT[0:DR, NCC * TSH + ti * P:
                                    NCC * TSH + (ti + 1) * P], tp[0:DR, :])
                # ---- AllGather the transposed shard ----
                drp = top.enter_context(
                    tc.tile_pool(name="dram", bufs=1, space="DRAM"))
                bin_ = drp.tile([KVL + DR, TSH], RT, tag="cc_in",
                                name="cc_in")
                bout = drp.tile([NCORES, KVL + DR, TSH], RT, tag="cc_out",
                                name="cc_out", addr_space="Shared")
                nc.gpsimd.dma_start(
                    bin_[0:KVL, :].rearrange("(cc p) f -> p cc f", p=P),
                    shT[:].rearrange("p (b f) -> p b f", b=5)[:, 0:NCC])
                nc.gpsimd.dma_start(
                    bin_[KVL:KVL + DR, :],
                    shT[0:DR, NCC * TSH:5 * TSH])
                nc.gpsimd.collective_compute(
                    "AllGather", mybir.AluOpType.bypass,
                    replica_groups=[list(range(NCORES))],
                    ins=[bin_[:].opt()],
                    outs=[bout[:].opt()])
                for cc in range(NCC):
                    nc.gpsimd.dma_start(
                        kvcT[cc][:].rearrange("p (g f) -> p g f", g=NCORES),
                        bout[:, cc * P:(cc + 1) * P, :].rearrange(
                            "g p f -> p g f"))
                nc.gpsimd.dma_start(
                    kpeT[:].rearrange("p (g f) -> p g f", g=NCORES),
                    bout[:, KVL:KVL + DR, :].rearrange("g p f -> p g f"))

            # ===== phase 1: q (+ kv if not sharded) projections ============
            kvc = kpe = None
            if not SHARD_KV:
                for q4 in range(4):
                    hd = slice(q4 * (NDC // 4), (q4 + 1) * (NDC // 4))
                    nc.sync.dma_start(
                        wkva_a[:].rearrange("p (d c) -> p d c",
                                            d=NDC)[:, hd],
                        d_wkva.ap().rearrange("(d p) c -> p d c",
                                              p=P)[:, hd])
                kvp = top.enter_context(tc.tile_pool(name="kv", bufs=NTT))
                kvc = [kvp.tile([P, KVL], RT, tag="kvc", name="kvc")
                       for _ in range(NTT)]
                kpp = top.enter_context(tc.tile_pool(name="kpe", bufs=NTT))
                kpe = [kpp.tile([P, DR], RT, tag="kpe", name="kpe")
                       for _ in range(NTT)]

            with ExitStack() as ph1:
                wrp = ph1.enter_context(tc.tile_pool(name="wres", bufs=1))
                xsl = ph1.enter_context(tc.tile_pool(name="xsl", bufs=6))
                xTj0 = d_xT.ap()[:, 0:SF].rearrange("(d p) f -> p d f", p=P)
                xh0 = [xsl.tile([P, 4 * SF], RT, tag="xsl", name="xsl")
                       for _ in range(4)]
                wqn_a = wrp.tile([P, NDC * NHC * DN], RT, tag="wqn",
                                 name="wqn")
                wqpr_a = wrp.tile([P, NDC * NHC * 32], RT, tag="wqpr",
                                  name="wqpr")
                wqpi_a = wrp.tile([P, NDC * NHC * 32], RT, tag="wqpi",
                                  name="wqpi")
                for q4 in range(4):
                    hd = slice(q4 * (NDC // 4), (q4 + 1) * (NDC // 4))
                    nc.sync.dma_start(
                        xh0[q4][:].rearrange("p (d f) -> p d f", d=4),
                        xTj0[:, 4 * q4:4 * (q4 + 1)])
                    nc.sync.dma_start(
                        wqn_a[:].rearrange("p (d c) -> p d c", d=NDC)[:, hd],
                        d_wqn.ap().rearrange("(d p) c -> p d c", p=P)[:, hd])
                    nc.sync.dma_start(
                        wqpr_a[:].rearrange("p (d c) -> p d c", d=NDC)[:, hd],
                        d_wqpr.ap().rearrange("(d p) c -> p d c", p=P)[:, hd])
                    nc.sync.dma_start(
                        wqpi_a[:].rearrange("p (d c) -> p d c", d=NDC)[:, hd],
                        d_wqpi.ap().rearrange("(d p) c -> p d c", p=P)[:, hd])
                if not SHARD_KV:
                    cna = wrp.tile([P, NTT * 32], F32, tag="cna", name="cna")
                    sna = wrp.tile([P, NTT * 32], F32, tag="sna", name="sna")
                    nc.sync.dma_start(
                        cna[:].rearrange("p (t k) -> p t k", t=NTT),
                        d_cosn.ap().rearrange("(t p) k -> p t k", p=P))
                    nc.sync.dma_start(
                        sna[:].rearrange("p (t k) -> p t k", t=NTT),
                        d_sinn.ap().rearrange("(t p) k -> p t k", p=P))

                rts = ph1.enter_context(tc.tile_pool(name="ropetmp", bufs=1))
                sqs = ph1.enter_context(tc.tile_pool(name="sqs", bufs=2))
                crs = ph1.enter_context(tc.tile_pool(name="crs", bufs=2))
                kct = ph1.enter_context(tc.tile_pool(name="kct", bufs=2))
                nrm = ph1.enter_context(tc.tile_pool(name="nrm", bufs=4))

                with tc.tile_pool(name="acc1", bufs=8, space="PSUM") as qac:
                    for j in range(NST):
                        js = slice(j * SF, (j + 1) * SF)
                        xTj = d_xT.ap()[:, js].rearrange(
                            "(d p) f -> p d f", p=P)
                        if j == 0:
                            xh = xh0
                        else:
                            xh = [xsl.tile([P, 4 * SF], RT, tag="xsl",
                                           name="xsl") for _ in range(4)]
                            for q4 in range(4):
                                nc.sync.dma_start(
                                    xh[q4][:].rearrange(
                                        "p (d f) -> p d f", d=4),
                                    xTj[:, 4 * q4:4 * (q4 + 1)])
                        # ---- q projections for this s block ----
                        pss = [qac.tile([P, SF], F32, tag="acc", name="acc")
                               for _ in range(NHC + 2)]
                        for d in range(NDC):
                            xs = xh[d // 4][:, (d % 4) * SF:(d % 4 + 1) * SF]
                            for h in range(NHC):
                                nc.tensor.matmul(
                                    pss[h][:],
                                    wqn_a[:, d * NHC * DN + h * DN:
                                          d * NHC * DN + (h + 1) * DN],
                                    xs,
                                    start=(d == 0), stop=(d == NDC - 1))
                            nc.tensor.matmul(
                                pss[NHC][:],
                                wqpr_a[:, d * P:(d + 1) * P], xs,
                                start=(d == 0), stop=(d == NDC - 1))
                            nc.tensor.matmul(
                                pss[NHC + 1][:],
                                wqpi_a[:, d * P:(d + 1) * P], xs,
                                start=(d == 0), stop=(d == NDC - 1))
                        for h in range(NHC):
                            nc.scalar.copy(qns[h][:, js], pss[h][:])
                        # rope rotation for q_pe (even=r, odd=i) off PSUM
                        t1 = rts.tile([P, SF], F32, tag="t1", name="t1")
                        t2 = rts.tile([P, SF], F32, tag="t2", name="t2")
                        ror = rts.tile([P, SF], F32, tag="ror", name="ror")
                        roi = rts.tile([P, SF], F32, tag="roi", name="roi")
                        cR = crs.tile([P, SF], F32, tag="cR", name="cR")
                        sR = crs.tile([P, SF], F32, tag="sR", name="sR")
                        nc.sync.dma_start(cR[:], d_cosr.ap()[:, js])
                        nc.sync.dma_start(sR[:], d_sinr.ap()[:, js])
                        nc.vector.tensor_mul(t1[:], pss[NHC][:], cR[:])
                        nc.vector.tensor_mul(t2[:], pss[NHC + 1][:], sR[:])
                        nc.vector.tensor_sub(ror[:], t1[:], t2[:])
                        nc.vector.tensor_mul(t1[:], pss[NHC][:], sR[:])
                        nc.vector.tensor_mul(t2[:], pss[NHC + 1][:], cR[:])
                        nc.vector.tensor_add(roi[:], t1[:], t2[:])
                        for h in range(NHC):
                            hs = slice(h * 32, (h + 1) * 32)
                            nc.vector.tensor_copy(
                                qps[h][0:32, js], ror[hs, :])
                            nc.vector.tensor_copy(
                                qps[h][32:64, js], roi[hs, :])
                        if SHARD_KV:
                            continue
                        # ---- kv projection for this t block (same x) ----
                        for ti in range(4):
                            psc = qac.tile([P, KVL], F32, tag="acc",
                                           name="acc")
                            psp = qac.tile([P, DR], F32, tag="acc",
                                           name="accp", padded_shape=[P, SF])
                            for d in range(NDC):
                                xs = xh[d // 4]
                                xtsl = xs[:, (d % 4) * SF + ti * P:
                                          (d % 4) * SF + (ti + 1) * P]
                                nc.tensor.matmul(
                                    psc[:], xtsl,
                                    wkva_a[:, d * (KVL + DR):
                                           d * (KVL + DR) + KVL],
                                    start=(d == 0), stop=(d == NDC - 1))
                                nc.tensor.matmul(
                                    psp[:], xtsl,
                                    wkva_a[:, d * (KVL + DR) + KVL:
                                           (d + 1) * (KVL + DR)],
                                    start=(d == 0), stop=(d == NDC - 1))
                            t = j * 4 + ti
                            sq = sqs.tile([P, KVL], F32, tag="sq", name="sq")
                            ss = nrm.tile([P, 1], F32, tag="ss", name="ss")
                            nc.scalar.activation(
                                sq[:], psc[:],
                                mybir.ActivationFunctionType.Square,
                                accum_out=ss[:])
                            rt_ = nrm.tile([P, 1], F32, tag="rt", name="rt")
                            nc.scalar.activation(
                                rt_[:], ss[:],
                                mybir.ActivationFunctionType.Sqrt,
                                bias=epsb[:], scale=1.0 / KVL)
                            ri = nrm.tile([P, 1], F32, tag="ri", name="ri")
                            nc.vector.reciprocal(ri[:], rt_[:])
                            nc.scalar.mul(kvc[t][:], psc[:], ri[:])
                            # k rope (deinterleave to [r(32) | i(32)])
                            cn = cna[:, t * 32:(t + 1) * 32]
                            sn = sna[:, t * 32:(t + 1) * 32]
                            pe = psp[:].rearrange(
                                "p (k two) -> p k two", two=2)
                            xr = pe[:, :, 0:1].rearrange(
                                "p k one -> p (k one)")
                            xi = pe[:, :, 1:2].rearrange(
                                "p k one -> p (k one)")
                            m1 = kct.tile([P, DR // 2], F32, tag="m1",
                                          name="m1")
                            m2 = kct.tile([P, DR // 2], F32, tag="m2",
                                          name="m2")
                            nc.vector.tensor_mul(m1[:], xr, cn)
                            nc.vector.tensor_mul(m2[:], xi, sn)
                            nc.vector.tensor_sub(kpe[t][:, 0:32], m1[:],
                                                 m2[:])
                            nc.vector.tensor_mul(m1[:], xr, sn)
                            nc.vector.tensor_mul(m2[:], xi, cn)
                            nc.vector.tensor_add(kpe[t][:, 32:64], m1[:],
                                                 m2[:])

            # ======== phase 2: transposes, K/V materialize, attention ======
            msp = top.enter_context(
                tc.tile_pool(name="msp", bufs=4, space="PSUM"))
            otp = top.enter_context(tc.tile_pool(name="oT", bufs=NHC))
            oTs = [otp.tile([DV, S], RT, tag="oT", name="oT")
                   for _ in range(NHC)]
            if not SHARD_KV:
                with tc.tile_pool(name="tpp", bufs=4, space="PSUM") as tpp:
                    for t in range(NTT):
                        ts_ = slice(t * P, (t + 1) * P)
                        for cc in range(NCC):
                            tp = tpp.tile([P, P], RT, tag="mspt",
                                          name="mspt")
                            nc.tensor.transpose(
                                tp[:], kvc[t][:, cc * P:(cc + 1) * P],
                                ident[:])
                            nc.scalar.copy(kvcT[cc][:, ts_], tp[:])
                        tp = tpp.tile([P, P], RT, tag="mspt", name="mspt")
                        nc.tensor.transpose(tp[0:DR, :], kpe[t][:], ident[:])
                        nc.scalar.copy(kpeT[:, ts_], tp[0:DR, :])

            # V for all 4 heads: V_all[t] = kv_c[t] @ wbv  -> [t(128), 4*DV]
            vap = top.enter_context(tc.tile_pool(name="vall", bufs=NTT))
            wbvp = top.enter_context(tc.tile_pool(name="wbv", bufs=1))
            wbv_a = wbvp.tile([P, NCC * NHC * DV], RT, tag="wbv",
                              name="wbv")
            nc.sync.dma_start(
                wbv_a[:].rearrange("p (cc f) -> p cc f", cc=NCC),
                d_wbv.ap().rearrange("(cc p) f -> p cc f", p=P))
            vall = [vap.tile([P, NHC * DV], RT, tag="vall", name="vall")
                    for _ in range(NTT)]
            for t in range(NTT):
                ts_ = slice(t * P, (t + 1) * P)
                ps = msp.tile([P, SF], F32, tag="msp", name="msp")
                for cc in range(NCC):
                    nc.tensor.matmul(
                        ps[:], kvcT[cc][:, ts_],
                        wbv_a[:, cc * NHC * DV:(cc + 1) * NHC * DV],
                        start=(cc == 0), stop=(cc == NCC - 1))
                nc.scalar.copy(vall[t][:], ps[:])

            with ExitStack() as ph2:
                wbkp = ph2.enter_context(tc.tile_pool(name="wbk", bufs=2))
                ktp = ph2.enter_context(tc.tile_pool(name="kt", bufs=2))
                etp = ph2.enter_context(tc.tile_pool(name="et", bufs=6))
                ohp = ph2.enter_context(
                    tc.tile_pool(name="ohp", bufs=2, space="PSUM"))
                dnp = ph2.enter_context(
                    tc.tile_pool(name="dn", bufs=2, space="PSUM"))
                dvp = ph2.enter_context(tc.tile_pool(name="dinv", bufs=2))

                for h in range(NHC):
                    # K_h^T = wbk_h^T(scaled) @ kv_c^T   [DN, S]
                    wbk = wbkp.tile([P, NCC * DN], RT, tag="wbk", name="wbk")
                    nc.sync.dma_start(
                        wbk[:].rearrange("p (cc f) -> p cc f", cc=NCC),
                        d_wbkT.ap()[h].rearrange("(cc p) f -> p cc f", p=P))
                    kT = ktp.tile([DN, S], RT, tag="kT", name="kT")
                    for j in range(NST):
                        js = slice(j * SF, (j + 1) * SF)
                        ps = msp.tile([P, SF], F32, tag="msp", name="msp")
                        for cc in range(NCC):
                            nc.tensor.matmul(
                                ps[:], wbk[:, cc * DN:(cc + 1) * DN],
                                kvcT[cc][:, js],
                                start=(cc == 0), stop=(cc == NCC - 1))
                        nc.scalar.copy(kT[:, js], ps[:])
                    for j in range(NST):
                        js = slice(j * SF, (j + 1) * SF)
                        oh = ohp.tile([P, SF], F32, tag="oh", name="oh")
                        dn = dnp.tile([1, SF], F32, tag="dn", name="dn")
                        ntt = 4 * j + 4
                        for t in range(ntt):
                            ts_ = slice(t * P, (t + 1) * P)
                            # causal narrowing: tile t only needs columns
                            # s >= t*128, i.e. local offset 128*(t-4j)
                            off = max(0, P * (t - 4 * j))
                            nf = SF - off
                            osl = slice(j * SF + off, (j + 1) * SF)
                            sc = msp.tile([P, SF], F32, tag="msp",
                                          name="msp")
                            nc.tensor.matmul(
                                sc[:, 0:nf], kT[:, ts_], qns[h][:, osl],
                                start=True, stop=False)
                            nc.tensor.matmul(
                                sc[:, 0:nf], kpeT[:, ts_], qps[h][:, osl],
                                start=False, stop=True)
                            e = etp.tile([P, SF], RT, tag="et", name="et")
                            nc.scalar.activation(
                                e[:, 0:nf], sc[:, 0:nf],
                                mybir.ActivationFunctionType.Exp,
                                scale=SCALE)
                            if t >= 4 * j:
                                # triangular mask on the first 128 columns
                                # of the diagonal tile (keep c >= p)
                                nc.gpsimd.affine_select(
                                    out=e[:, 0:P], in_=e[:, 0:P],
                                    compare_op=mybir.AluOpType.is_ge,
                                    fill=0.0, base=0,
                                    pattern=[[1, P]],
                                    channel_multiplier=-1)
                            nc.tensor.matmul(
                                dn[:, off:SF], ones_c[:], e[:, 0:nf],
                                start=(t == 0), stop=(t == ntt - 1))
                            nc.tensor.matmul(
                                oh[:, off:SF],
                                vall[t][:, h * DV:(h + 1) * DV],
                                e[:, 0:nf], start=(t == 0),
                                stop=(t == ntt - 1))
                        di = dvp.tile([1, SF], RT, tag="di", name="di")
                        with nc.allow_low_precision(
                                reason="bf16 1/denom is within tolerance"):
                            nc.vector.reciprocal(di[:], dn[:])
                        db = dvp.tile([P, SF], RT, tag="db", name="db")
                        nc.gpsimd.partition_broadcast(db[:], di[:])
                        nc.vector.tensor_mul(oTs[h][:, js], oh[:], db[:])

            # ============ phase 3: output projection (partial) =============
            with ExitStack() as ph3:
                wop = ph3.enter_context(tc.tile_pool(name="wo", bufs=NHC))
                otg = ph3.enter_context(tc.tile_pool(name="ost", bufs=3))
                wos = [wop.tile([DV, DIM], RT, tag="wo", name="wo")
                       for _ in range(NHC)]
                for h in range(NHC):
                    nc.sync.dma_start(
                        wos[h][:], d_wo.ap()[h * DV:(h + 1) * DV, :])
                for d in range(NDC):
                    ds_ = slice(d * P, (d + 1) * P)
                    obig = otg.tile([P, S], F16, tag="ost", name="ost")
                    for j in range(NST):
                        js = slice(j * SF, (j + 1) * SF)
                        ps = msp.tile([P, SF], F32, tag="msp", name="msp")
                        for h in range(NHC):
                            nc.tensor.matmul(
                                ps[:], wos[h][:, ds_], oTs[h][:, js],
                                start=(h == 0), stop=(h == NHC - 1))
                        nc.vector.tensor_copy(obig[:, js], ps[:])
                    nc.sync.dma_start(out[ds_, :], obig[:])

    nc.compile()
    return nc


def prep_inputs(x, wq_w, wkv_a_w, wkv_b_w, kv_norm_w, wo_w,
                freqs_cos, freqs_sin):
    """Host-side sharding/layout prep. Returns per-core input maps."""
    import ml_dtypes
    bf16 = ml_dtypes.bfloat16
    x = np.ascontiguousarray(np.asarray(x, np.float32).reshape(S, DIM))
    xT = np.ascontiguousarray(x.T).astype(bf16)
    wq = np.asarray(wq_w, np.float32).reshape(DIM, NH, DN + DR)
    wkva = np.ascontiguousarray(np.asarray(wkv_a_w, np.float32)).astype(bf16)
    wkvb = np.asarray(wkv_b_w, np.float32)
    knw = np.asarray(kv_norm_w, np.float32)
    wo = np.asarray(wo_w, np.float32)
    cos = np.asarray(freqs_cos, np.float32)
    sin = np.asarray(freqs_sin, np.float32)
    cosR = np.ascontiguousarray(np.tile(cos.T, (NHC, 1)))  # [128, S]
    sinR = np.ascontiguousarray(np.tile(sin.T, (NHC, 1)))

    maps = []
    for c in range(NCORES):
        hs = list(range(NHC * c, NHC * (c + 1)))
        wq_n = np.ascontiguousarray(
            wq[:, hs, :DN].reshape(DIM, NHC * DN)).astype(bf16)
        wq_pr = np.ascontiguousarray(
            wq[:, hs, DN + 0::2].reshape(DIM, NHC * 32)).astype(bf16)
        wq_pi = np.ascontiguousarray(
            wq[:, hs, DN + 1::2].reshape(DIM, NHC * 32)).astype(bf16)
        # fold kv_norm weight into the absorbed weights
        # wbkT[h] = (wbk_h * knw).T  [KVL, DN]
        wbkT = np.stack([
            np.ascontiguousarray(
                (wkvb[h * (DN + DV):h * (DN + DV) + DN, :] * knw[None, :]).T)
            for h in hs]).astype(bf16)                      # [4, 512, 128]
        # wbv_all = concat_h (wbv_h^T * knw[:,None])  [KVL, 4*DV]
        wbv_all = np.concatenate(
            [np.ascontiguousarray(
                wkvb[h * (DN + DV) + DN:(h + 1) * (DN + DV), :].T)
             * knw[:, None] for h in hs], axis=1).astype(bf16)  # [512, 512]
        wo_c = np.ascontiguousarray(
            np.concatenate([wo[h * DV:(h + 1) * DV, :]
                            for h in hs])).astype(bf16)
        m = {
            "xT": xT, "wq_n": wq_n, "wq_pr": wq_pr, "wq_pi": wq_pi,
            "wkv_a": wkva, "wbkT": np.ascontiguousarray(wbkT),
            "wbv_all": np.ascontiguousarray(wbv_all), "wo_c": wo_c,
            "cosR": cosR, "sinR": sinR,
        }
        if SHARD_KV:
            m["x_own"] = np.ascontiguousarray(
                xT[:, c * TSH:(c + 1) * TSH])
            # cos/sin for own 2 t-tiles, [128, 2*32]
            co = cos[c * TSH:(c + 1) * TSH].reshape(2, P, DR // 2)
            si = sin[c * TSH:(c + 1) * TSH].reshape(2, P, DR // 2)
            m["cos_o"] = np.ascontiguousarray(
                co.transpose(1, 0, 2).reshape(P, 2 * 32))
            m["sin_o"] = np.ascontiguousarray(
                si.transpose(1, 0, 2).reshape(P, 2 * 32))
        else:
            m["cos_n"] = cos
            m["sin_n"] = sin
        maps.append(m)
    return maps


def kernel(x, wq_w, wkv_a_w, wkv_b_w, kv_norm_w, wo_w,
           freqs_cos, freqs_sin, start_pos):
    assert int(start_pos) == 0
    maps = prep_inputs(x, wq_w, wkv_a_w, wkv_b_w, kv_norm_w, wo_w,
                       freqs_cos, freqs_sin)
    nc = build_nc()
    res = run_bass_kernel_spmd(nc, maps, list(range(NCORES)))
    acc = np.zeros((DIM, S), np.float64)
    for c in range(NCORES):
        acc += res.results[c]["outT"].astype(np.float64)
    return np.ascontiguousarray(acc.T).astype(np.float32).reshape(1, S, DIM)
